# revision 13
# baseline (speedup 1.0000x reference)
"""Trainium2 Bass kernel for nn_CBFLayer (batch CBF-QP safety filter).

Contract: kernel(u_nom, obs) takes FULL inputs (numpy), returns FULL output.
Internally: pure data-parallel shard of the batch across 8 NeuronCores.

Math (per sample, exact KKT of the QP  min |u-u_nom|^2 + LAM*s^2
s.t. a@u <= b+s, |u|^2 <= 1, s >= 0, with a = -G, G = 2*p_rel):
  u = (u_nom + t*G) * rho,  rho = rsqrt(max(|u_nom + t*G|^2, 1))
with multiplier t per KKT case: t=0 (case-1 feasible: P'*rn + b >= 0,
rn = rsqrt(max(N,1))), t2 = -(P'+b)/S' (case 2, valid iff
|u2|^2 = N + (b^2-P'^2)/S' <= 1; the t2>=0 guard is redundant there),
or the circle root
  t3 = -(P' + |C|*b*rsqrt(max(|C|/LAM, S'-b^2) + eps)) / S'
max'ed with the deep-infeasible branch t = LAM*relu(-(b+sqrt(S'))).
S'=|G|^2, P'=G.u, C=GyUx-GxUy, b = S'/2 - 2 - G.v.  The t-multiplier
chain runs in the NEGATED domain (tneg = -t) so every op is a plain
tensor_tensor or an immediate tensor_scalar.

Engine findings baked in (measured on HW):
- DVE tensor_tensor bf16 = 2x mode; tensor_scalar with IMMEDIATE
  scalars = 4x; scalar_tensor_tensor = 1x - avoided entirely;
- GpSimd elementwise REMOVED: its SBUF port contends with the DVE and
  slows concurrent Vector ops 2-6x (large net loss);
- rsqrt/square/abs on ScalarE via the abs_reciprocal_sqrt_and_small
  table (plain Rsqrt activation is blocked by bass; the
  Abs_reciprocal_sqrt variant is equivalent for nonneg inputs);
- uneven tiles [1536,1536,1024] cut per-instruction overhead (58-cyc
  DVE issue cost + semaphores) 25% vs four 1024 tiles; scratch tags
  are aliased by lifetime to fit SBUF.
"""

import numpy as np
from ml_dtypes import bfloat16

B = 4194304
NCORES = 8
BC = B // NCORES            # 524288 samples per core
P = 128
NPER = BC // P              # 4096 samples per partition
WS = [1024, 1536, 1536]     # tile widths (samples per partition)
OFFS = [0, 1024, 2560]
NT = len(WS)

LAM = 10000.0
TOL = 1e-6

_CACHE = {}


def _build():
    import bass_rust as _bass_rust
    import concourse.bacc as bacc
    import concourse.mybir as mybir
    from concourse.tile import TileContext
    from concourse.hw_specs import get_activation_tables

    F32 = mybir.dt.float32
    BF16 = mybir.dt.bfloat16
    U16 = mybir.dt.uint16
    OP = mybir.AluOpType
    AF = mybir.ActivationFunctionType

    class _PinnedBacc(bacc.Bacc):
        """Activation-table chooser only sees abs_reciprocal_sqrt_and_small
        (list order preserved so act_func_set_id indices stay aligned)."""

        def insert_act_table_loads(self):
            has_activation = any(
                isinstance(i, mybir.InstActivation)
                for b in self.main_func.blocks
                for i in b.instructions
            )
            if not has_activation:
                return
            tables = [
                (k, v if k == "abs_reciprocal_sqrt_and_small" else set())
                for k, v in get_activation_tables(self.m.arch).items()
            ]
            _bass_rust.insert_act_table_loads(self, tables)

    nc = _PinnedBacc("TRN2", target_bir_lowering=False, debug=False)
    pk_in = nc.dram_tensor("pk", [P, NPER * 6], BF16, kind="ExternalInput").ap()
    cst_in = nc.dram_tensor("cst", [P, 8], F32, kind="ExternalInput").ap()
    out_d = nc.dram_tensor("out", [P, NPER * 2], BF16, kind="ExternalOutput").ap()

    # scratch tags aliased by lifetime (the aliasee is dead before the
    # aliaser is written) so the bufs=1 pool fits SBUF at width 1536
    ALIAS = {"d": "Nm", "df": "S4m2", "ddn": "rn", "ddr": "num",
             "n2": "PRN", "cb": "psq", "cbr": "bsq2", "X": "ac2",
             "tmain": "acr", "ta1": "ff", "talt": "rsq"}

    with TileContext(nc) as tc:
        with (
            tc.tile_pool(name="io", bufs=2) as io,
            tc.tile_pool(name="wk", bufs=2) as wk,
            tc.tile_pool(name="ck", bufs=1) as ck,
        ):
            cst = ck.tile([P, 8], F32, tag="cst", name="cst")
            nc.sync.dma_start(out=cst[:], in_=cst_in[:])
            for j, value in enumerate([0.0, -1.0, 1.0, -0.5 * TOL, 1e-30]):
                nc.const_aps.aps[(F32, value)] = cst[:, j:j + 1]

            def tt(out, a, b, op):
                nc.vector.tensor_tensor(out, a, b, op)

            def ts(out, a, s1, op0, s2=None, op1=None):
                if op1 is None:
                    nc.vector.tensor_scalar(out, a, s1, None, op0)
                else:
                    nc.vector.tensor_scalar(out, a, s1, s2, op0, op1)

            def act(out, a, func, scale=1.0, bias=0.0):
                nc.scalar.activation(out, a, func, bias=bias, scale=scale)

            def T(name, n, dt=BF16):
                return ck.tile([P, n], dt, tag=ALIAS.get(name, name),
                               name=name)

            def TW(name, n, dt=BF16):
                return wk.tile([P, n], dt, tag=name, name=name)

            def stage_dma(i):
                w, off = WS[i], OFFS[i]
                st = {"w": w, "off": off}
                o_t = io.tile([P, 2 * w], BF16, tag="o_t")
                pk_t = io.tile([P, 6 * w], BF16, tag="pk_t")
                base = 6 * off
                if i == 0:
                    for c0, c1 in ((0, 2), (2, 4), (4, 6)):
                        nc.sync.dma_start(out=pk_t[:, c0 * w:c1 * w],
                                          in_=pk_in[:, base + c0 * w:
                                                    base + c1 * w])
                else:
                    nc.sync.dma_start(out=pk_t[:],
                                      in_=pk_in[:, base:base + 6 * w])
                st["pk_t"], st["o_t"] = pk_t, o_t
                st["gb"], st["ub"] = pk_t[:, 0:2 * w], pk_t[:, 2 * w:4 * w]
                return st

            def stage_a(i, st):
                # pk blocks: [Gx | Gy | ux | uy | vx | vy], G = 2*p_rel
                w = st["w"]
                pk_t = st["pk_t"]
                gb = st["gb"]
                sq4 = T("sq4", 4 * w)
                act(sq4[:], pk_t[:, 0:4 * w], AF.Square)
                # cross first (needs only blocks 0-3)
                cu0 = T("cu0", w)
                tt(cu0[:], pk_t[:, w:2 * w], pk_t[:, 2 * w:3 * w], OP.mult)
                cu1 = T("cu1", w)
                tt(cu1[:], pk_t[:, 0:w], pk_t[:, 3 * w:4 * w], OP.mult)
                C = TW("C", w)
                tt(C[:], cu0[:], cu1[:], OP.subtract)
                st["C"] = C
                # big4 = bcast[Gx|Gy] * [ux|uy|vx|vy] -> P' = G.u, VD2 = G.v
                big4 = T("big4", 4 * w)
                tt(big4[:].rearrange("p (a b) -> p a b", a=2),
                   gb.rearrange("p (o b) -> p o b", o=1).broadcast_to(
                       [P, 2, 2 * w]),
                   pk_t[:, 2 * w:6 * w].rearrange("p (a b) -> p a b", a=2),
                   OP.mult)
                PV = TW("PV", 2 * w)
                bv = big4[:].rearrange("p (a b) -> p a b", a=4)
                tt(PV[:].rearrange("p (a b) -> p a b", a=2),
                   bv[:, 0::2, :], bv[:, 1::2, :], OP.add)
                st["PV"] = PV
                # SN last on V (depends on the S-engine squares)
                SN = TW("SN", 2 * w)
                ev = sq4[:].rearrange("p (a b) -> p a b", a=4)
                tt(SN[:].rearrange("p (a b) -> p a b", a=2),
                   ev[:, 0::2, :], ev[:, 1::2, :], OP.add)
                st["SN"] = SN

            def solve(i, st):
                w = st["w"]
                SN, PV, C = st["SN"], st["PV"], st["C"]
                S_ = SN[:, 0:w]
                N_ = SN[:, w:2 * w]
                P_ = PV[:, 0:w]
                V2 = PV[:, w:2 * w]
                # seeds: launch S-engine rsqrt/squares early
                rsq = T("rsq", w); act(rsq[:], S_, AF.Abs_reciprocal_sqrt)
                Nm = T("Nm", w); ts(Nm[:], N_, 1.0, OP.max)
                rn = T("rn", w); act(rn[:], Nm[:], AF.Abs_reciprocal_sqrt)
                S4m2 = T("S4m2", w)
                act(S4m2[:], S_, AF.Copy, scale=0.5, bias=-2.0)
                bb = T("bb", w); tt(bb[:], S4m2[:], V2, OP.subtract)
                bsq2 = T("bsq2", w); act(bsq2[:], bb[:], AF.Square)
                psq = T("psq", w); act(psq[:], P_, AF.Square)
                ac2 = T("ac2", w); act(ac2[:], C[:], AF.Abs, scale=1.0 / LAM)
                acr = T("acr", w)
                ts(acr[:].bitcast(U16), C[:].bitcast(U16), 0x7FFF,
                   OP.bitwise_and)
                rS = T("rS", w); tt(rS[:], rsq[:], rsq[:], OP.mult)
                sqSp = T("sqSp", w); act(sqSp[:], rS[:], AF.Abs_reciprocal_sqrt)
                num = T("num", w); tt(num[:], P_, bb[:], OP.add)
                # case 1 feasibility: ff = P'*rn + b; nf1n = -(ff < 0)
                PRN = T("PRN", w); tt(PRN[:], P_, rn[:], OP.mult)
                ff = T("ff", w); tt(ff[:], PRN[:], bb[:], OP.add)
                nf1n = T("nf1n", w)
                ts(nf1n[:], ff[:], -0.5 * TOL, OP.is_lt, -1.0, OP.mult)
                # case 2 (negated): m2 = -t2 = (P'+b)/S'
                m2 = T("m2", w); tt(m2[:], num[:], rS[:], OP.mult)
                # case 3 pole: df = max(|C|/LAM, S'-b^2), rr = rsqrt(df+eps)
                d = T("d", w); tt(d[:], S_, bsq2[:], OP.subtract)
                df = T("df", w); tt(df[:], ac2[:], d[:], OP.max)
                rr = T("rr", w)
                act(rr[:], df[:], AF.Abs_reciprocal_sqrt, bias=1e-30)
                # |u2|^2 = N + (b^2 - P'^2)/S' <= 1 gates case 2
                ddn = T("ddn", w); tt(ddn[:], bsq2[:], psq[:], OP.subtract)
                ddr = T("ddr", w); tt(ddr[:], ddn[:], rS[:], OP.mult)
                n2 = T("n2", w); tt(n2[:], N_, ddr[:], OP.add)
                ok2 = T("ok2", w); ts(ok2[:], n2[:], 1.0, OP.is_le)
                # case 3 (negated): tmain = (P' + |C|*b*rr) * rS
                cb = T("cb", w); tt(cb[:], acr[:], bb[:], OP.mult)
                cbr = T("cbr", w); tt(cbr[:], cb[:], rr[:], OP.mult)
                X = T("X", w); tt(X[:], P_, cbr[:], OP.add)
                tmain = T("tmain", w); tt(tmain[:], X[:], rS[:], OP.mult)
                # deep infeasible (negated): talt = min(LAM*(b+sqrt(S')), 0)
                ta1 = T("ta1", w); tt(ta1[:], bb[:], sqSp[:], OP.add)
                talt = T("talt", w); ts(talt[:], ta1[:], LAM, OP.mult,
                                        0.0, OP.min)
                # select: tneg = min(tmain, talt); ok2 -> m2; t = tneg * nf1n
                t = TW("t", w); tt(t[:], tmain[:], talt[:], OP.min)
                nc.vector.copy_predicated(t[:], ok2[:].bitcast(U16), m2[:])
                tt(t[:], t[:], nf1n[:], OP.mult)
                st["t"] = t

            def vw2(ap, h0, h1):  # [h0:h1] sample-slice of an x|y pair
                return ap.rearrange("p (o b) -> p o b", o=2)[:, :, h0:h1]

            def solve2a(st, h0, h1):
                w = st["w"]
                gb, ub, t = st["gb"], st["ub"], st["t"]
                axy = T("axy", 2 * w)
                sxy = st["sxy"] = T("sxy", 2 * w)
                sq2 = st["sq2"] = T("sq2", 2 * w)
                hw = h1 - h0
                tt(vw2(axy[:], h0, h1),
                   t[:, h0:h1].rearrange(
                       "p (o b) -> p o b", o=1).broadcast_to([P, 2, hw]),
                   vw2(gb, h0, h1), OP.mult)
                tt(vw2(sxy[:], h0, h1), vw2(ub, h0, h1), vw2(axy[:], h0, h1),
                   OP.add)
                act(vw2(sq2[:], h0, h1), vw2(sxy[:], h0, h1), AF.Square)

            def solve2b(i, st, h0, h1):
                w, off = st["w"], st["off"]
                o_t, sxy, sq2 = st["o_t"], st["sxy"], st["sq2"]
                nn = T("nn", w)
                nnm = T("nnm", w)
                rho = T("rho", w)
                hw = h1 - h0
                tt(nn[:, h0:h1], sq2[:, h0:h1], sq2[:, w + h0:w + h1], OP.add)
                ts(nnm[:, h0:h1], nn[:, h0:h1], 1.0, OP.max)
                act(rho[:, h0:h1], nnm[:, h0:h1], AF.Abs_reciprocal_sqrt)
                tt(vw2(o_t, h0, h1), vw2(sxy[:], h0, h1),
                   rho[:, h0:h1].rearrange(
                       "p (o b) -> p o b", o=1).broadcast_to([P, 2, hw]),
                   OP.mult)
                ob = 2 * off
                nc.sync.dma_start(
                    out=out_d[:, ob:ob + 2 * w].rearrange(
                        "p (o b) -> p o b", o=2)[:, :, h0:h1],
                    in_=vw2(o_t, h0, h1))

            sts = {0: stage_dma(0)}
            stage_a(0, sts[0])
            for i in range(NT):
                if i + 1 < NT:
                    sts[i + 1] = stage_dma(i + 1)
                solve(i, sts[i])
                st = sts.pop(i)
                if i + 1 < NT:
                    # sq2 (S-engine) overlaps the next tile's stage_a V-ops
                    solve2a(st, 0, st["w"])
                    stage_a(i + 1, sts[i + 1])
                    solve2b(i, st, 0, st["w"])
                else:
                    # last tile: two half-width passes hide trailing
                    # S-engine latencies against remaining V work
                    w = st["w"]
                    for h0, h1 in ((0, w // 2), (w // 2, w)):
                        solve2a(st, h0, h1)
                        solve2b(i, st, h0, h1)
    nc.compile()
    return nc


def _get_nc():
    if "nc" not in _CACHE:
        _CACHE["nc"] = _build()
    return _CACHE["nc"]


def _run(u_nom: np.ndarray, obs: np.ndarray, trace: bool = False):
    from concourse.bass_utils import run_bass_kernel_spmd

    u_nom = np.asarray(u_nom, dtype=np.float32)
    obs = np.asarray(obs, dtype=np.float32)

    nc = _get_nc()
    cstv = np.tile(np.array([0.0, -1.0, 1.0, -0.5 * TOL, 1e-30,
                             0.0, 0.0, 0.0], dtype=np.float32), (P, 1))
    in_maps = []
    for c in range(NCORES):
        s = slice(c * BC, (c + 1) * BC)
        uc = u_nom[s].reshape(P, NPER, 2).astype(bfloat16)
        oc = obs[s].reshape(P, NPER, 6)
        gx = (2.0 * oc[:, :, 2]).astype(bfloat16)
        gy = (2.0 * oc[:, :, 3]).astype(bfloat16)
        # clamp exact-zero Gx so S' = Gx^2+Gy^2 > 0 on device
        gx = np.where(np.abs(gx.astype(np.float32)) < 1e-18,
                      np.float32(1e-18), gx.astype(np.float32)).astype(bfloat16)
        blocks = [gx, gy, uc[:, :, 0], uc[:, :, 1],
                  oc[:, :, 4].astype(bfloat16), oc[:, :, 5].astype(bfloat16)]
        # blocks per tile: [Gx|Gy|ux|uy|vx|vy] each of width w
        pk = np.concatenate(
            [np.concatenate([b[:, off:off + w] for b in blocks], axis=1)
             for off, w in zip(OFFS, WS)], axis=1)
        in_maps.append({"pk": np.ascontiguousarray(pk),
                        "cst": np.ascontiguousarray(cstv)})
    res = run_bass_kernel_spmd(nc, in_maps, core_ids=list(range(NCORES)),
                               trace=trace)
    out = np.empty((B, 2), dtype=np.float32)
    for c in range(NCORES):
        r = np.asarray(res.results[c]["out"]).view(bfloat16).astype(np.float32)
        r = r.reshape(P, NPER * 2)
        oc = np.empty((P, NPER, 2), dtype=np.float32)
        for off, w in zip(OFFS, WS):
            blk = r[:, 2 * off:2 * off + 2 * w].reshape(P, 2, w)
            oc[:, off:off + w, :] = np.transpose(blk, (0, 2, 1))
        out[c * BC:(c + 1) * BC] = oc.reshape(BC, 2)
    return out, res


def kernel(u_nom: np.ndarray, obs: np.ndarray) -> np.ndarray:
    return _run(u_nom, obs)[0]


if __name__ == "__main__":
    rng = np.random.default_rng(0)
    u = rng.standard_normal((B, 2), dtype=np.float32)
    o = rng.standard_normal((B, 6), dtype=np.float32)
    r = kernel(u, o)
    print(r.shape, r.dtype, r[:4])


# revision 14
# speedup vs baseline: 1.0113x; 1.0113x over previous
"""Trainium2 Bass kernel for nn_CBFLayer (batch CBF-QP safety filter).

Contract: kernel(u_nom, obs) takes FULL inputs (numpy), returns FULL output.
Internally: pure data-parallel shard of the batch across 8 NeuronCores.

Math (per sample, exact KKT of the QP  min |u-u_nom|^2 + LAM*s^2
s.t. a@u <= b+s, |u|^2 <= 1, s >= 0, with a = -G, G = 2*p_rel):
  u = (u_nom + t*G) * rho,  rho = rsqrt(max(|u_nom + t*G|^2, 1))
with multiplier t per KKT case: t=0 (case-1 feasible: P'*rn + b >= 0,
rn = rsqrt(max(N,1))), t2 = -(P'+b)/S' (case 2, valid iff t2>=0 and
|u2|^2 = N + (b^2-P'^2)/S' <= 1), or the circle root
  t3 = -(P' + |C|*b*rsqrt(max(|C|/LAM, S'-b^2) + eps)) / S'
max'ed with the deep-infeasible branch t = LAM*relu(-(b+sqrt(S'))).
S'=|G|^2, P'=G.u, C=GyUx-GxUy, b = S'/2 - 2 - G.v.  The t-multiplier
chain runs in the NEGATED domain (tneg = -t) so every op is a plain
tensor_tensor or an immediate tensor_scalar.

Engine findings baked in (measured on HW):
- DVE tensor_tensor bf16 = 2x mode (594ns/KC); tensor_scalar with
  IMMEDIATE scalars = 4x (336ns); scalar_tensor_tensor = 1x - avoided;
- GpSimd elementwise REMOVED entirely: its SBUF port contends with the
  DVE and slows concurrent Vector ops 2-6x (net large loss);
- all rsqrt/square/abs on ScalarE via the abs_reciprocal_sqrt_and_small
  table (Rsqrt/Reciprocal activations are blocked by bass; the
  Abs_reciprocal_sqrt variant is equivalent for nonneg inputs).
"""

import numpy as np
from ml_dtypes import bfloat16

B = 4194304
NCORES = 8
BC = B // NCORES            # 524288 samples per core
P = 128
NPER = BC // P              # 4096 samples per partition
KC = 1024                   # compute-tile samples per partition
NT = NPER // KC             # tiles per core

LAM = 10000.0
TOL = 1e-6

_CACHE = {}


def _build():
    import bass_rust as _bass_rust
    import concourse.bacc as bacc
    import concourse.mybir as mybir
    from concourse.tile import TileContext
    from concourse.hw_specs import get_activation_tables

    F32 = mybir.dt.float32
    BF16 = mybir.dt.bfloat16
    U16 = mybir.dt.uint16
    OP = mybir.AluOpType
    AF = mybir.ActivationFunctionType

    class _PinnedBacc(bacc.Bacc):
        """Activation-table chooser only sees abs_reciprocal_sqrt_and_small
        (list order preserved so act_func_set_id indices stay aligned)."""

        def insert_act_table_loads(self):
            has_activation = any(
                isinstance(i, mybir.InstActivation)
                for b in self.main_func.blocks
                for i in b.instructions
            )
            if not has_activation:
                return
            tables = [
                (k, v if k == "abs_reciprocal_sqrt_and_small" else set())
                for k, v in get_activation_tables(self.m.arch).items()
            ]
            _bass_rust.insert_act_table_loads(self, tables)

    nc = _PinnedBacc("TRN2", target_bir_lowering=False, debug=False)
    pk_in = nc.dram_tensor("pk", [P, NPER * 6], BF16, kind="ExternalInput").ap()
    cst_in = nc.dram_tensor("cst", [P, 8], F32, kind="ExternalInput").ap()
    out_d = nc.dram_tensor("out", [P, NPER * 2], BF16, kind="ExternalOutput").ap()

    with TileContext(nc) as tc:
        with (
            tc.tile_pool(name="io", bufs=2) as io,
            tc.tile_pool(name="wk", bufs=2) as wk,
            tc.tile_pool(name="ck", bufs=1) as ck,
        ):
            cst = ck.tile([P, 8], F32, tag="cst", name="cst")
            nc.sync.dma_start(out=cst[:], in_=cst_in[:])
            for j, value in enumerate([0.0, -1.0, 1.0, -0.5 * TOL, 1e-30]):
                nc.const_aps.aps[(F32, value)] = cst[:, j:j + 1]

            def tt(out, a, b, op):
                nc.vector.tensor_tensor(out, a, b, op)

            def ts(out, a, s1, op0, s2=None, op1=None):
                if op1 is None:
                    nc.vector.tensor_scalar(out, a, s1, None, op0)
                else:
                    nc.vector.tensor_scalar(out, a, s1, s2, op0, op1)

            def act(out, a, func, scale=1.0, bias=0.0):
                nc.scalar.activation(out, a, func, bias=bias, scale=scale)

            def T(name, n, dt=BF16):
                return ck.tile([P, n], dt, tag=name, name=name)

            def TW(name, n, dt=BF16):
                return wk.tile([P, n], dt, tag=name, name=name)

            def stage_dma(i):
                st = {}
                o_t = io.tile([P, 2 * KC], BF16, tag="o_t")
                pk_t = io.tile([P, 6 * KC], BF16, tag="pk_t")
                if i == 0:
                    for c0, c1 in ((0, 2), (2, 4), (4, 6)):
                        nc.sync.dma_start(out=pk_t[:, c0 * KC:c1 * KC],
                                          in_=pk_in[:, c0 * KC:c1 * KC])
                else:
                    nc.sync.dma_start(out=pk_t[:],
                                      in_=pk_in[:, i * 6 * KC:(i + 1) * 6 * KC])
                st["pk_t"], st["o_t"] = pk_t, o_t
                st["gb"], st["ub"] = pk_t[:, 0:2 * KC], pk_t[:, 2 * KC:4 * KC]
                return st

            def stage_a(i, st):
                # pk blocks: [Gx | Gy | ux | uy | vx | vy], G = 2*p_rel
                pk_t = st["pk_t"]
                gb = st["gb"]
                # squares of [Gx|Gy|ux|uy]; S-engine first so it starts on
                # DMA-land while V finishes the previous tile
                sq4 = T("sq4", 4 * KC)
                act(sq4[:], pk_t[:, 0:4 * KC], AF.Square)
                # cross first (needs only blocks 0-3 -> starts on chunk 2)
                cu0 = T("cu0", KC)
                tt(cu0[:], pk_t[:, KC:2 * KC], pk_t[:, 2 * KC:3 * KC], OP.mult)
                cu1 = T("cu1", KC)
                tt(cu1[:], pk_t[:, 0:KC], pk_t[:, 3 * KC:4 * KC], OP.mult)
                C = TW("C", KC)
                tt(C[:], cu0[:], cu1[:], OP.subtract)
                st["C"] = C
                # big4 = bcast[Gx|Gy] * [ux|uy|vx|vy] -> P' = G.u, VD2 = G.v
                big4 = T("big4", 4 * KC)
                tt(big4[:].rearrange("p (a b) -> p a b", a=2),
                   gb.rearrange("p (o b) -> p o b", o=1).broadcast_to([P, 2, 2 * KC]),
                   pk_t[:, 2 * KC:6 * KC].rearrange("p (a b) -> p a b", a=2),
                   OP.mult)
                PV = TW("PV", 2 * KC)
                bv = big4[:].rearrange("p (a b) -> p a b", a=4)
                tt(PV[:].rearrange("p (a b) -> p a b", a=2),
                   bv[:, 0::2, :], bv[:, 1::2, :], OP.add)
                st["PV"] = PV
                # SN last on V (depends on the S-engine squares)
                SN = TW("SN", 2 * KC)
                ev = sq4[:].rearrange("p (a b) -> p a b", a=4)
                tt(SN[:].rearrange("p (a b) -> p a b", a=2),
                   ev[:, 0::2, :], ev[:, 1::2, :], OP.add)
                st["SN"] = SN

            def solve(i, st):
                SN, PV, C = st["SN"], st["PV"], st["C"]
                S_ = SN[:, 0:KC]
                N_ = SN[:, KC:2 * KC]
                P_ = PV[:, 0:KC]
                V2 = PV[:, KC:2 * KC]
                # seeds: launch S-engine rsqrt/squares early
                rsq = T("rsq", KC); act(rsq[:], S_, AF.Abs_reciprocal_sqrt)
                Nm = T("Nm", KC); ts(Nm[:], N_, 1.0, OP.max)
                rn = T("rn", KC); act(rn[:], Nm[:], AF.Abs_reciprocal_sqrt)
                S4m2 = T("S4m2", KC)
                act(S4m2[:], S_, AF.Copy, scale=0.5, bias=-2.0)
                bb = T("bb", KC); tt(bb[:], S4m2[:], V2, OP.subtract)
                bsq2 = T("bsq2", KC); act(bsq2[:], bb[:], AF.Square)
                psq = T("psq", KC); act(psq[:], P_, AF.Square)
                ac2 = T("ac2", KC); act(ac2[:], C[:], AF.Abs, scale=1.0 / LAM)
                acr = T("acr", KC)
                ts(acr[:].bitcast(U16), C[:].bitcast(U16), 0x7FFF, OP.bitwise_and)
                rS = T("rS", KC); tt(rS[:], rsq[:], rsq[:], OP.mult)
                sqSp = T("sqSp", KC); act(sqSp[:], rS[:], AF.Abs_reciprocal_sqrt)
                num = T("num", KC); tt(num[:], P_, bb[:], OP.add)
                # case 1 feasibility: ff = P'*rn + b; nf1n = -(ff < 0)
                PRN = T("PRN", KC); tt(PRN[:], P_, rn[:], OP.mult)
                ff = T("ff", KC); tt(ff[:], PRN[:], bb[:], OP.add)
                nf1n = T("nf1n", KC)
                ts(nf1n[:], ff[:], -0.5 * TOL, OP.is_lt, -1.0, OP.mult)
                # case 2 (negated): m2 = -t2 = (P'+b)/S'
                m2 = T("m2", KC); tt(m2[:], num[:], rS[:], OP.mult)
                # case 3 pole: df = max(|C|/LAM, S'-b^2), rr = rsqrt(df+eps)
                d = T("d", KC); tt(d[:], S_, bsq2[:], OP.subtract)
                df = T("df", KC); tt(df[:], ac2[:], d[:], OP.max)
                rr = T("rr", KC); act(rr[:], df[:], AF.Abs_reciprocal_sqrt, bias=1e-30)
                # |u2|^2 = N + (b^2 - P'^2)/S' <= 1 gates case 2 (the t2>=0
                # guard is redundant: infeasible & n2<=1 implies t2>=0)
                ddn = T("ddn", KC); tt(ddn[:], bsq2[:], psq[:], OP.subtract)
                ddr = T("ddr", KC); tt(ddr[:], ddn[:], rS[:], OP.mult)
                n2 = T("n2", KC); tt(n2[:], N_, ddr[:], OP.add)
                ok2 = T("ok2", KC); ts(ok2[:], n2[:], 1.0, OP.is_le)
                # case 3 (negated): tmain_neg = (P' + |C|*b*rr) * rS
                cb = T("cb", KC); tt(cb[:], acr[:], bb[:], OP.mult)
                cbr = T("cbr", KC); tt(cbr[:], cb[:], rr[:], OP.mult)
                X = T("X", KC); tt(X[:], P_, cbr[:], OP.add)
                tmain = T("tmain", KC); tt(tmain[:], X[:], rS[:], OP.mult)
                # deep infeasible (negated): talt_neg = min(LAM*(b+sqrt(S')), 0)
                ta1 = T("ta1", KC); tt(ta1[:], bb[:], sqSp[:], OP.add)
                talt = T("talt", KC); ts(talt[:], ta1[:], LAM, OP.mult, 0.0, OP.min)
                # select: tneg = min(tmain, talt); ok2 -> m2; t = tneg * nf1n
                t = TW("t", KC); tt(t[:], tmain[:], talt[:], OP.min)
                nc.vector.copy_predicated(t[:], ok2[:].bitcast(U16), m2[:])
                tt(t[:], t[:], nf1n[:], OP.mult)
                st["t"] = t

            def solve2(i, st):
                gb, ub, o_t, t = st["gb"], st["ub"], st["o_t"], st["t"]
                axy = T("axy", 2 * KC)
                sxy = T("sxy", 2 * KC)
                sq2 = T("sq2", 2 * KC)
                nn = T("nn", KC)
                nnm = T("nnm", KC)
                rho = T("rho", KC)
                # last tile: two half-width passes so the trailing S-engine
                # latencies (sq2, rho) pipeline against V instead of hanging
                # off the end of the kernel
                halves = ((0, KC // 2), (KC // 2, KC)) if i == NT - 1 \
                    else ((0, KC),)
                for h0, h1 in halves:
                    w = h1 - h0

                    def vw(ap, n):  # [h0:h1] sample-slice of an x|y pair tile
                        return ap.rearrange(
                            "p (o b) -> p o b", o=2)[:, :, h0:h1]

                    tt(vw(axy[:], KC),
                       t[:, h0:h1].rearrange(
                           "p (o b) -> p o b", o=1).broadcast_to([P, 2, w]),
                       vw(gb, KC), OP.mult)
                    tt(vw(sxy[:], KC), vw(ub, KC), vw(axy[:], KC), OP.add)
                    act(vw(sq2[:], KC), vw(sxy[:], KC), AF.Square)
                    tt(nn[:, h0:h1], sq2[:, h0:h1], sq2[:, KC + h0:KC + h1],
                       OP.add)
                    ts(nnm[:, h0:h1], nn[:, h0:h1], 1.0, OP.max)
                    act(rho[:, h0:h1], nnm[:, h0:h1], AF.Abs_reciprocal_sqrt)
                    tt(vw(o_t, KC), vw(sxy[:], KC),
                       rho[:, h0:h1].rearrange(
                           "p (o b) -> p o b", o=1).broadcast_to([P, 2, w]),
                       OP.mult)
                    nc.sync.dma_start(
                        out=out_d[:].rearrange(
                            "p (i o b) -> p i o b", i=NT, o=2)[:, i, :, h0:h1],
                        in_=vw(o_t, KC))

            sts = {0: stage_dma(0)}
            stage_a(0, sts[0])
            for i in range(NT):
                if i + 1 < NT:
                    sts[i + 1] = stage_dma(i + 1)
                solve(i, sts[i])
                if i + 1 < NT:
                    stage_a(i + 1, sts[i + 1])
                solve2(i, sts.pop(i))
    nc.compile()
    return nc


def _get_nc():
    if "nc" not in _CACHE:
        _CACHE["nc"] = _build()
    return _CACHE["nc"]


def _run(u_nom: np.ndarray, obs: np.ndarray, trace: bool = False):
    from concourse.bass_utils import run_bass_kernel_spmd

    u_nom = np.asarray(u_nom, dtype=np.float32)
    obs = np.asarray(obs, dtype=np.float32)

    nc = _get_nc()
    in_maps = []
    for c in range(NCORES):
        s = slice(c * BC, (c + 1) * BC)
        uc = u_nom[s].reshape(P, NT, KC, 2).astype(bfloat16)
        oc = obs[s].reshape(P, NT, KC, 6)
        gx = (2.0 * oc[:, :, :, 2]).astype(bfloat16)
        gy = (2.0 * oc[:, :, :, 3]).astype(bfloat16)
        # clamp exact-zero Gx so S' = Gx^2+Gy^2 > 0 on device (no stt floor)
        gx = np.where(np.abs(gx.astype(np.float32)) < 1e-18,
                      np.float32(1e-18), gx.astype(np.float32)).astype(bfloat16)
        # blocks: [Gx | Gy | ux | uy | vx | vy], G = 2*p_rel
        pk = np.stack(
            [gx, gy,
             uc[:, :, :, 0], uc[:, :, :, 1],
             oc[:, :, :, 4].astype(bfloat16), oc[:, :, :, 5].astype(bfloat16)],
            axis=2).reshape(P, NPER * 6)
        cstv = np.tile(np.array([0.0, -1.0, 1.0, -0.5 * TOL, 1e-30,
                                 0.0, 0.0, 0.0], dtype=np.float32), (P, 1))
        in_maps.append({"pk": np.ascontiguousarray(pk),
                        "cst": np.ascontiguousarray(cstv)})
    res = run_bass_kernel_spmd(nc, in_maps, core_ids=list(range(NCORES)),
                               trace=trace)
    out = np.empty((B, 2), dtype=np.float32)
    for c in range(NCORES):
        r = np.asarray(res.results[c]["out"]).view(bfloat16).astype(np.float32)
        r = r.reshape(P, NT, 2, KC)
        out[c * BC:(c + 1) * BC] = np.transpose(r, (0, 1, 3, 2)).reshape(BC, 2)
    return out, res


def kernel(u_nom: np.ndarray, obs: np.ndarray) -> np.ndarray:
    return _run(u_nom, obs)[0]


if __name__ == "__main__":
    rng = np.random.default_rng(0)
    u = rng.standard_normal((B, 2), dtype=np.float32)
    o = rng.standard_normal((B, 6), dtype=np.float32)
    r = kernel(u, o)
    print(r.shape, r.dtype, r[:4])


# revision 15
# speedup vs baseline: 1.0842x; 1.0721x over previous
"""Trainium2 Bass kernel for nn_CBFLayer (batch CBF-QP safety filter).

Contract: kernel(u_nom, obs) takes FULL inputs (numpy), returns FULL output.
Internally: pure data-parallel shard of the batch across 8 NeuronCores.

Math (per sample, exact KKT of the QP  min |u-u_nom|^2 + LAM*s^2
s.t. a@u <= b+s, |u|^2 <= 1, s >= 0, with a = -G, G = 2*p_rel):
  u = (u_nom + t*G) * rho,  rho = rsqrt(max(|u_nom + t*G|^2, 1))
with multiplier t per KKT case: t=0 (case-1 feasible: P'*rn + b >= 0,
rn = rsqrt(max(N,1))), t2 = -(P'+b)/S' (case 2, valid iff t2>=0 and
|u2|^2 = N + (b^2-P'^2)/S' <= 1), or the circle root
  t3 = -(P' + |C|*b*rsqrt(max(|C|/LAM, S'-b^2) + eps)) / S'
max'ed with the deep-infeasible branch t = LAM*relu(-(b+sqrt(S'))).
S'=|G|^2, P'=G.u, C=GyUx-GxUy, b = S'/2 - 2 - G.v.  The t-multiplier
chain runs in the NEGATED domain (tneg = -t) so every op is a plain
tensor_tensor or an immediate tensor_scalar.

Engine findings baked in (measured on HW):
- DVE tensor_tensor bf16 = 2x mode (594ns/KC); tensor_scalar with
  IMMEDIATE scalars = 4x (336ns); scalar_tensor_tensor = 1x - avoided;
- GpSimd elementwise REMOVED entirely: its SBUF port contends with the
  DVE and slows concurrent Vector ops 2-6x (net large loss);
- all rsqrt/square/abs on ScalarE via the abs_reciprocal_sqrt_and_small
  table (Rsqrt/Reciprocal activations are blocked by bass; the
  Abs_reciprocal_sqrt variant is equivalent for nonneg inputs).
"""

import numpy as np
from ml_dtypes import bfloat16

B = 4194304
NCORES = 8
BC = B // NCORES            # 524288 samples per core
P = 128
NPER = BC // P              # 4096 samples per partition
KC = 1024                   # compute-tile samples per partition
NT = NPER // KC             # tiles per core

LAM = 10000.0
TOL = 1e-6

_CACHE = {}


def _build():
    import bass_rust as _bass_rust
    import concourse.bacc as bacc
    import concourse.mybir as mybir
    from concourse.tile import TileContext
    from concourse.hw_specs import get_activation_tables

    F32 = mybir.dt.float32
    BF16 = mybir.dt.bfloat16
    U16 = mybir.dt.uint16
    OP = mybir.AluOpType
    AF = mybir.ActivationFunctionType

    class _PinnedBacc(bacc.Bacc):
        """Activation-table chooser only sees abs_reciprocal_sqrt_and_small
        (list order preserved so act_func_set_id indices stay aligned)."""

        def insert_act_table_loads(self):
            has_activation = any(
                isinstance(i, mybir.InstActivation)
                for b in self.main_func.blocks
                for i in b.instructions
            )
            if not has_activation:
                return
            tables = [
                (k, v if k == "abs_reciprocal_sqrt_and_small" else set())
                for k, v in get_activation_tables(self.m.arch).items()
            ]
            _bass_rust.insert_act_table_loads(self, tables)

    nc = _PinnedBacc("TRN2", target_bir_lowering=False, debug=False)
    pk_in = nc.dram_tensor("pk", [P, NPER * 6], BF16, kind="ExternalInput").ap()
    cst_in = nc.dram_tensor("cst", [P, 8], F32, kind="ExternalInput").ap()
    out_d = nc.dram_tensor("out", [P, NPER * 2], BF16, kind="ExternalOutput").ap()

    with TileContext(nc) as tc:
        with (
            tc.tile_pool(name="io", bufs=2) as io,
            tc.tile_pool(name="wk", bufs=2) as wk,
            tc.tile_pool(name="ck", bufs=1) as ck,
        ):
            cst = ck.tile([P, 8], F32, tag="cst", name="cst")
            nc.sync.dma_start(out=cst[:], in_=cst_in[:])
            for j, value in enumerate([0.0, -1.0, 1.0, -0.5 * TOL, 1e-30]):
                nc.const_aps.aps[(F32, value)] = cst[:, j:j + 1]

            def tt(out, a, b, op):
                nc.vector.tensor_tensor(out, a, b, op)

            def ts(out, a, s1, op0, s2=None, op1=None):
                if op1 is None:
                    nc.vector.tensor_scalar(out, a, s1, None, op0)
                else:
                    nc.vector.tensor_scalar(out, a, s1, s2, op0, op1)

            def act(out, a, func, scale=1.0, bias=0.0):
                nc.scalar.activation(out, a, func, bias=bias, scale=scale)

            def T(name, n, dt=BF16):
                return ck.tile([P, n], dt, tag=name, name=name)

            def TW(name, n, dt=BF16):
                return wk.tile([P, n], dt, tag=name, name=name)

            def stage_dma(i):
                st = {}
                o_t = io.tile([P, 2 * KC], BF16, tag="o_t")
                pk_t = io.tile([P, 6 * KC], BF16, tag="pk_t")
                if i == 0:
                    for c0, c1 in ((0, 2), (2, 4), (4, 6)):
                        nc.sync.dma_start(out=pk_t[:, c0 * KC:c1 * KC],
                                          in_=pk_in[:, c0 * KC:c1 * KC])
                else:
                    nc.sync.dma_start(out=pk_t[:],
                                      in_=pk_in[:, i * 6 * KC:(i + 1) * 6 * KC])
                st["pk_t"], st["o_t"] = pk_t, o_t
                st["gb"], st["ub"] = pk_t[:, 0:2 * KC], pk_t[:, 2 * KC:4 * KC]
                return st

            def stage_a(i, st):
                # pk blocks: [Gx | Gy | ux | uy | vx | vy], G = 2*p_rel
                pk_t = st["pk_t"]
                gb = st["gb"]
                # squares of [Gx|Gy|ux|uy]; S-engine first so it starts on
                # DMA-land while V finishes the previous tile
                sq4 = T("sq4", 4 * KC)
                act(sq4[:], pk_t[:, 0:4 * KC], AF.Square)
                # cross first (needs only blocks 0-3 -> starts on chunk 2)
                cu0 = T("cu0", KC)
                tt(cu0[:], pk_t[:, KC:2 * KC], pk_t[:, 2 * KC:3 * KC], OP.mult)
                cu1 = T("cu1", KC)
                tt(cu1[:], pk_t[:, 0:KC], pk_t[:, 3 * KC:4 * KC], OP.mult)
                C = TW("C", KC)
                tt(C[:], cu0[:], cu1[:], OP.subtract)
                st["C"] = C
                # big4 = bcast[Gx|Gy] * [ux|uy|vx|vy] -> P' = G.u, VD2 = G.v
                big4 = T("big4", 4 * KC)
                tt(big4[:].rearrange("p (a b) -> p a b", a=2),
                   gb.rearrange("p (o b) -> p o b", o=1).broadcast_to([P, 2, 2 * KC]),
                   pk_t[:, 2 * KC:6 * KC].rearrange("p (a b) -> p a b", a=2),
                   OP.mult)
                PV = TW("PV", 2 * KC)
                bv = big4[:].rearrange("p (a b) -> p a b", a=4)
                tt(PV[:].rearrange("p (a b) -> p a b", a=2),
                   bv[:, 0::2, :], bv[:, 1::2, :], OP.add)
                st["PV"] = PV
                # SN last on V (depends on the S-engine squares)
                SN = TW("SN", 2 * KC)
                ev = sq4[:].rearrange("p (a b) -> p a b", a=4)
                tt(SN[:].rearrange("p (a b) -> p a b", a=2),
                   ev[:, 0::2, :], ev[:, 1::2, :], OP.add)
                st["SN"] = SN

            def solve(i, st):
                SN, PV, C = st["SN"], st["PV"], st["C"]
                S_ = SN[:, 0:KC]
                N_ = SN[:, KC:2 * KC]
                P_ = PV[:, 0:KC]
                V2 = PV[:, KC:2 * KC]
                # seeds: launch S-engine rsqrt/squares early
                rsq = T("rsq", KC); act(rsq[:], S_, AF.Abs_reciprocal_sqrt)
                S4m2 = T("S4m2", KC)
                act(S4m2[:], S_, AF.Copy, scale=0.5, bias=-2.0)
                bb = T("bb", KC); tt(bb[:], S4m2[:], V2, OP.subtract)
                bsq2 = T("bsq2", KC); act(bsq2[:], bb[:], AF.Square)
                psq = T("psq", KC); act(psq[:], P_, AF.Square)
                ac2 = T("ac2", KC); act(ac2[:], C[:], AF.Abs, scale=1.0 / LAM)
                acr = T("acr", KC)
                ts(acr[:].bitcast(U16), C[:].bitcast(U16), 0x7FFF, OP.bitwise_and)
                rS = T("rS", KC); tt(rS[:], rsq[:], rsq[:], OP.mult)
                sqSp = T("sqSp", KC); act(sqSp[:], rS[:], AF.Abs_reciprocal_sqrt)
                num = T("num", KC); tt(num[:], P_, bb[:], OP.add)
                # case 2 (negated): m2 = -t2 = (P'+b)/S'
                m2 = T("m2", KC); tt(m2[:], num[:], rS[:], OP.mult)
                # case 3 pole: df = max(|C|/LAM, S'-b^2), rr = rsqrt(df+eps)
                d = T("d", KC); tt(d[:], S_, bsq2[:], OP.subtract)
                df = T("df", KC); tt(df[:], ac2[:], d[:], OP.max)
                rr = T("rr", KC); act(rr[:], df[:], AF.Abs_reciprocal_sqrt, bias=1e-30)
                # |u2|^2 = N + (b^2 - P'^2)/S' <= 1 gates case 2 (the t2>=0
                # guard is redundant: infeasible & n2<=1 implies t2>=0)
                ddn = T("ddn", KC); tt(ddn[:], bsq2[:], psq[:], OP.subtract)
                ddr = T("ddr", KC); tt(ddr[:], ddn[:], rS[:], OP.mult)
                n2 = T("n2", KC); tt(n2[:], N_, ddr[:], OP.add)
                ok2 = T("ok2", KC); ts(ok2[:], n2[:], 1.0, OP.is_le)
                # case 3 (negated): tmain_neg = (P' + |C|*b*rr) * rS
                cb = T("cb", KC); tt(cb[:], acr[:], bb[:], OP.mult)
                cbr = T("cbr", KC); tt(cbr[:], cb[:], rr[:], OP.mult)
                X = T("X", KC); tt(X[:], P_, cbr[:], OP.add)
                tmain = T("tmain", KC); tt(tmain[:], X[:], rS[:], OP.mult)
                # deep infeasible (negated): talt_neg = min(LAM*(b+sqrt(S')), 0)
                ta1 = T("ta1", KC); tt(ta1[:], bb[:], sqSp[:], OP.add)
                talt = T("talt", KC); ts(talt[:], ta1[:], LAM, OP.mult)
                # select: tneg = min(tmain, talt); ok2 -> m2; the final clamp
                # t = relu(-tneg) subsumes the case-1 feasibility test
                # (feasible samples have tneg >= 0 up to bf16 noise)
                t = TW("t", KC); tt(t[:], tmain[:], talt[:], OP.min)
                nc.vector.copy_predicated(t[:], ok2[:].bitcast(U16), m2[:])
                ts(t[:], t[:], -1.0, OP.mult, 0.0, OP.max)
                st["t"] = t

            def solve2(i, st):
                gb, ub, o_t, t = st["gb"], st["ub"], st["o_t"], st["t"]
                axy = T("axy", 2 * KC)
                sxy = T("sxy", 2 * KC)
                sq2 = T("sq2", 2 * KC)
                nn = T("nn", KC)
                nnm = T("nnm", KC)
                rho = T("rho", KC)
                # last tile: two half-width passes so the trailing S-engine
                # latencies (sq2, rho) pipeline against V instead of hanging
                # off the end of the kernel
                halves = ((0, KC // 2), (KC // 2, KC)) if i == NT - 1 \
                    else ((0, KC),)
                for h0, h1 in halves:
                    w = h1 - h0

                    def vw(ap, n):  # [h0:h1] sample-slice of an x|y pair tile
                        return ap.rearrange(
                            "p (o b) -> p o b", o=2)[:, :, h0:h1]

                    tt(vw(axy[:], KC),
                       t[:, h0:h1].rearrange(
                           "p (o b) -> p o b", o=1).broadcast_to([P, 2, w]),
                       vw(gb, KC), OP.mult)
                    tt(vw(sxy[:], KC), vw(ub, KC), vw(axy[:], KC), OP.add)
                    act(vw(sq2[:], KC), vw(sxy[:], KC), AF.Square)
                    tt(nn[:, h0:h1], sq2[:, h0:h1], sq2[:, KC + h0:KC + h1],
                       OP.add)
                    ts(nnm[:, h0:h1], nn[:, h0:h1], 1.0, OP.max)
                    act(rho[:, h0:h1], nnm[:, h0:h1], AF.Abs_reciprocal_sqrt)
                    tt(vw(o_t, KC), vw(sxy[:], KC),
                       rho[:, h0:h1].rearrange(
                           "p (o b) -> p o b", o=1).broadcast_to([P, 2, w]),
                       OP.mult)
                    nc.sync.dma_start(
                        out=out_d[:].rearrange(
                            "p (i o b) -> p i o b", i=NT, o=2)[:, i, :, h0:h1],
                        in_=vw(o_t, KC))

            sts = {0: stage_dma(0)}
            stage_a(0, sts[0])
            for i in range(NT):
                if i + 1 < NT:
                    sts[i + 1] = stage_dma(i + 1)
                solve(i, sts[i])
                if i + 1 < NT:
                    stage_a(i + 1, sts[i + 1])
                solve2(i, sts.pop(i))
    nc.compile()
    return nc


def _get_nc():
    if "nc" not in _CACHE:
        _CACHE["nc"] = _build()
    return _CACHE["nc"]


def _run(u_nom: np.ndarray, obs: np.ndarray, trace: bool = False):
    from concourse.bass_utils import run_bass_kernel_spmd

    u_nom = np.asarray(u_nom, dtype=np.float32)
    obs = np.asarray(obs, dtype=np.float32)

    nc = _get_nc()
    in_maps = []
    for c in range(NCORES):
        s = slice(c * BC, (c + 1) * BC)
        uc = u_nom[s].reshape(P, NT, KC, 2).astype(bfloat16)
        oc = obs[s].reshape(P, NT, KC, 6)
        gx = (2.0 * oc[:, :, :, 2]).astype(bfloat16)
        gy = (2.0 * oc[:, :, :, 3]).astype(bfloat16)
        # clamp exact-zero Gx so S' = Gx^2+Gy^2 > 0 on device (no stt floor)
        gx = np.where(np.abs(gx.astype(np.float32)) < 1e-18,
                      np.float32(1e-18), gx.astype(np.float32)).astype(bfloat16)
        # blocks: [Gx | Gy | ux | uy | vx | vy], G = 2*p_rel
        pk = np.stack(
            [gx, gy,
             uc[:, :, :, 0], uc[:, :, :, 1],
             oc[:, :, :, 4].astype(bfloat16), oc[:, :, :, 5].astype(bfloat16)],
            axis=2).reshape(P, NPER * 6)
        cstv = np.tile(np.array([0.0, -1.0, 1.0, -0.5 * TOL, 1e-30,
                                 0.0, 0.0, 0.0], dtype=np.float32), (P, 1))
        in_maps.append({"pk": np.ascontiguousarray(pk),
                        "cst": np.ascontiguousarray(cstv)})
    res = run_bass_kernel_spmd(nc, in_maps, core_ids=list(range(NCORES)),
                               trace=trace)
    out = np.empty((B, 2), dtype=np.float32)
    for c in range(NCORES):
        r = np.asarray(res.results[c]["out"]).view(bfloat16).astype(np.float32)
        r = r.reshape(P, NT, 2, KC)
        out[c * BC:(c + 1) * BC] = np.transpose(r, (0, 1, 3, 2)).reshape(BC, 2)
    return out, res


def kernel(u_nom: np.ndarray, obs: np.ndarray) -> np.ndarray:
    return _run(u_nom, obs)[0]


if __name__ == "__main__":
    rng = np.random.default_rng(0)
    u = rng.standard_normal((B, 2), dtype=np.float32)
    o = rng.standard_normal((B, 6), dtype=np.float32)
    r = kernel(u, o)
    print(r.shape, r.dtype, r[:4])


# revision 16
# speedup vs baseline: 1.0998x; 1.0144x over previous
"""Trainium2 Bass kernel for nn_CBFLayer (batch CBF-QP safety filter).

Contract: kernel(u_nom, obs) takes FULL inputs (numpy), returns FULL output.
Internally: pure data-parallel shard of the batch across 8 NeuronCores.

Math (per sample, exact KKT of the QP  min |u-u_nom|^2 + LAM*s^2
s.t. a@u <= b+s, |u|^2 <= 1, s >= 0, with a = -G, G = 2*p_rel):
  u = (u_nom + t*G) * rho,  rho = rsqrt(max(|u_nom + t*G|^2, 1))
with multiplier t per KKT case: t=0 (case-1 feasible: P'*rn + b >= 0,
rn = rsqrt(max(N,1))), t2 = -(P'+b)/S' (case 2, valid iff t2>=0 and
|u2|^2 = N + (b^2-P'^2)/S' <= 1), or the circle root
  t3 = -(P' + |C|*b*rsqrt(max(|C|/LAM, S'-b^2) + eps)) / S'
max'ed with the deep-infeasible branch t = LAM*relu(-(b+sqrt(S'))).
S'=|G|^2, P'=G.u, C=GyUx-GxUy, b = S'/2 - 2 - G.v.  The t-multiplier
chain runs in the NEGATED domain (tneg = -t) so every op is a plain
tensor_tensor or an immediate tensor_scalar.

Engine findings baked in (measured on HW):
- DVE tensor_tensor bf16 = 2x mode (594ns/KC); tensor_scalar with
  IMMEDIATE scalars = 4x (336ns); scalar_tensor_tensor = 1x - avoided;
- GpSimd elementwise REMOVED entirely: its SBUF port contends with the
  DVE and slows concurrent Vector ops 2-6x (net large loss);
- all rsqrt/square/abs on ScalarE via the abs_reciprocal_sqrt_and_small
  table (Rsqrt/Reciprocal activations are blocked by bass; the
  Abs_reciprocal_sqrt variant is equivalent for nonneg inputs).
"""

import numpy as np
from ml_dtypes import bfloat16

B = 4194304
NCORES = 8
BC = B // NCORES            # 524288 samples per core
P = 128
NPER = BC // P              # 4096 samples per partition
KC = 1024                   # compute-tile samples per partition
NT = NPER // KC             # tiles per core

LAM = 10000.0
TOL = 1e-6

_CACHE = {}


def _build():
    import bass_rust as _bass_rust
    import concourse.bacc as bacc
    import concourse.mybir as mybir
    from concourse.tile import TileContext
    from concourse.hw_specs import get_activation_tables

    F32 = mybir.dt.float32
    BF16 = mybir.dt.bfloat16
    U16 = mybir.dt.uint16
    OP = mybir.AluOpType
    AF = mybir.ActivationFunctionType

    class _PinnedBacc(bacc.Bacc):
        """Activation-table chooser only sees abs_reciprocal_sqrt_and_small
        (list order preserved so act_func_set_id indices stay aligned)."""

        def insert_act_table_loads(self):
            has_activation = any(
                isinstance(i, mybir.InstActivation)
                for b in self.main_func.blocks
                for i in b.instructions
            )
            if not has_activation:
                return
            tables = [
                (k, v if k == "abs_reciprocal_sqrt_and_small" else set())
                for k, v in get_activation_tables(self.m.arch).items()
            ]
            _bass_rust.insert_act_table_loads(self, tables)

    nc = _PinnedBacc("TRN2", target_bir_lowering=False, debug=False)
    pk_in = nc.dram_tensor("pk", [P, NPER * 6], BF16, kind="ExternalInput").ap()
    cst_in = nc.dram_tensor("cst", [P, 8], F32, kind="ExternalInput").ap()
    out_d = nc.dram_tensor("out", [P, NPER * 2], BF16, kind="ExternalOutput").ap()

    with TileContext(nc) as tc:
        with (
            tc.tile_pool(name="io", bufs=2) as io,
            tc.tile_pool(name="wk", bufs=2) as wk,
            tc.tile_pool(name="ck", bufs=1) as ck,
        ):
            cst = ck.tile([P, 8], F32, tag="cst", name="cst")
            for j, value in enumerate([0.0, -1.0, 1.0, -0.5 * TOL, 1e-30,
                                       1e-38]):
                nc.const_aps.aps[(F32, value)] = cst[:, j:j + 1]

            def tt(out, a, b, op):
                nc.vector.tensor_tensor(out, a, b, op)

            def ts(out, a, s1, op0, s2=None, op1=None):
                if op1 is None:
                    nc.vector.tensor_scalar(out, a, s1, None, op0)
                else:
                    nc.vector.tensor_scalar(out, a, s1, s2, op0, op1)

            def act(out, a, func, scale=1.0, bias=0.0):
                nc.scalar.activation(out, a, func, bias=bias, scale=scale)

            def T(name, n, dt=BF16):
                return ck.tile([P, n], dt, tag=name, name=name)

            def TW(name, n, dt=BF16):
                return wk.tile([P, n], dt, tag=name, name=name)

            def stage_dma(i):
                st = {}
                o_t = io.tile([P, 2 * KC], BF16, tag="o_t")
                pk_t = io.tile([P, 6 * KC], BF16, tag="pk_t")
                if i == 0:
                    for c0, c1 in ((0, 3), (3, 4), (4, 6)):
                        nc.sync.dma_start(out=pk_t[:, c0 * KC:c1 * KC],
                                          in_=pk_in[:, c0 * KC:c1 * KC])
                else:
                    nc.sync.dma_start(out=pk_t[:],
                                      in_=pk_in[:, i * 6 * KC:(i + 1) * 6 * KC])
                st["pk_t"], st["o_t"] = pk_t, o_t
                st["gb"], st["ub"] = pk_t[:, 0:2 * KC], pk_t[:, 2 * KC:4 * KC]
                return st

            def stage_a(i, st):
                # pk blocks: [Gx | Gy | ux | uy | vx | vy], G = 2*p_rel
                pk_t = st["pk_t"]
                gb = st["gb"]
                # squares of [Gx|Gy|ux|uy]; S-engine first so it starts on
                # DMA-land while V finishes the previous tile
                sq4 = T("sq4", 4 * KC)
                act(sq4[:], pk_t[:, 0:4 * KC], AF.Square)
                # cross first (needs only blocks 0-3 -> starts on chunk 2)
                cu0 = T("cu0", KC)
                tt(cu0[:], pk_t[:, KC:2 * KC], pk_t[:, 2 * KC:3 * KC], OP.mult)
                cu1 = T("cu1", KC)
                tt(cu1[:], pk_t[:, 0:KC], pk_t[:, 3 * KC:4 * KC], OP.mult)
                C = TW("C", KC)
                tt(C[:], cu0[:], cu1[:], OP.subtract)
                st["C"] = C
                # big4 = bcast[Gx|Gy] * [ux|uy|vx|vy] -> P' = G.u, VD2 = G.v
                big4 = T("big4", 4 * KC)
                tt(big4[:].rearrange("p (a b) -> p a b", a=2),
                   gb.rearrange("p (o b) -> p o b", o=1).broadcast_to([P, 2, 2 * KC]),
                   pk_t[:, 2 * KC:6 * KC].rearrange("p (a b) -> p a b", a=2),
                   OP.mult)
                PV = TW("PV", 2 * KC)
                bv = big4[:].rearrange("p (a b) -> p a b", a=4)
                tt(PV[:].rearrange("p (a b) -> p a b", a=2),
                   bv[:, 0::2, :], bv[:, 1::2, :], OP.add)
                st["PV"] = PV
                # SN last on V (depends on the S-engine squares)
                SN = TW("SN", 2 * KC)
                ev = sq4[:].rearrange("p (a b) -> p a b", a=4)
                tt(SN[:].rearrange("p (a b) -> p a b", a=2),
                   ev[:, 0::2, :], ev[:, 1::2, :], OP.add)
                st["SN"] = SN

            def solve(i, st):
                SN, PV, C = st["SN"], st["PV"], st["C"]
                S_ = SN[:, 0:KC]
                N_ = SN[:, KC:2 * KC]
                P_ = PV[:, 0:KC]
                V2 = PV[:, KC:2 * KC]
                # seeds: launch S-engine rsqrt/squares early
                rsq = T("rsq", KC); act(rsq[:], S_, AF.Abs_reciprocal_sqrt)
                S4m2 = T("S4m2", KC)
                act(S4m2[:], S_, AF.Copy, scale=0.5, bias=-2.0)
                bb = T("bb", KC); tt(bb[:], S4m2[:], V2, OP.subtract)
                bsq2 = T("bsq2", KC); act(bsq2[:], bb[:], AF.Square)
                psq = T("psq", KC); act(psq[:], P_, AF.Square)
                ac2 = T("ac2", KC); act(ac2[:], C[:], AF.Abs, scale=1.0 / LAM)
                rS = T("rS", KC); tt(rS[:], rsq[:], rsq[:], OP.mult)
                sqSp = T("sqSp", KC); act(sqSp[:], rS[:], AF.Abs_reciprocal_sqrt)
                num = T("num", KC); tt(num[:], P_, bb[:], OP.add)
                # case 2 (negated): m2 = -t2 = (P'+b)/S'
                m2 = T("m2", KC); tt(m2[:], num[:], rS[:], OP.mult)
                # case 3 pole: df = max(|C|/LAM, S'-b^2), rr = rsqrt(df+eps)
                d = T("d", KC); tt(d[:], S_, bsq2[:], OP.subtract)
                df = T("df", KC); tt(df[:], ac2[:], d[:], OP.max)
                # rr = LAM*rsqrt(df+1e-30), via the act's free input affine
                rr = T("rr", KC)
                act(rr[:], df[:], AF.Abs_reciprocal_sqrt, scale=1e-8, bias=1e-38)
                # |u2|^2 = N + (b^2 - P'^2)/S' <= 1 gates case 2 (the t2>=0
                # guard is redundant: infeasible & n2<=1 implies t2>=0)
                ddn = T("ddn", KC); tt(ddn[:], bsq2[:], psq[:], OP.subtract)
                ddr = T("ddr", KC); tt(ddr[:], ddn[:], rS[:], OP.mult)
                n2 = T("n2", KC); tt(n2[:], N_, ddr[:], OP.add)
                ok2 = T("ok2", KC); ts(ok2[:], n2[:], 1.0, OP.is_le)
                # case 3 (negated): tmain_neg = (P' + |C|*b*rr) * rS
                cb = T("cb", KC); tt(cb[:], ac2[:], bb[:], OP.mult)
                cbr = T("cbr", KC); tt(cbr[:], cb[:], rr[:], OP.mult)
                X = T("X", KC); tt(X[:], P_, cbr[:], OP.add)
                tmain = T("tmain", KC); tt(tmain[:], X[:], rS[:], OP.mult)
                # deep infeasible (negated): talt_neg = min(LAM*(b+sqrt(S')), 0)
                ta1 = T("ta1", KC); tt(ta1[:], bb[:], sqSp[:], OP.add)
                talt = T("talt", KC); ts(talt[:], ta1[:], LAM, OP.mult)
                # select: tneg = min(tmain, talt); ok2 -> m2; the final clamp
                # t = relu(-tneg) subsumes the case-1 feasibility test
                # (feasible samples have tneg >= 0 up to bf16 noise)
                t = TW("t", KC); tt(t[:], tmain[:], talt[:], OP.min)
                nc.vector.copy_predicated(t[:], ok2[:].bitcast(U16), m2[:])
                ts(t[:], t[:], -1.0, OP.mult, 0.0, OP.max)
                st["t"] = t

            def solve2(i, st):
                gb, ub, o_t, t = st["gb"], st["ub"], st["o_t"], st["t"]
                axy = T("axy", 2 * KC)
                sxy = T("sxy", 2 * KC)
                sq2 = T("sq2", 2 * KC)
                nn = T("nn", KC)
                nnm = T("nnm", KC)
                rho = T("rho", KC)
                # last tile: two half-width passes so the trailing S-engine
                # latencies (sq2, rho) pipeline against V instead of hanging
                # off the end of the kernel
                halves = ((0, KC // 2), (KC // 2, KC)) if i == NT - 1 \
                    else ((0, KC),)
                for h0, h1 in halves:
                    w = h1 - h0

                    def vw(ap, n):  # [h0:h1] sample-slice of an x|y pair tile
                        return ap.rearrange(
                            "p (o b) -> p o b", o=2)[:, :, h0:h1]

                    tt(vw(axy[:], KC),
                       t[:, h0:h1].rearrange(
                           "p (o b) -> p o b", o=1).broadcast_to([P, 2, w]),
                       vw(gb, KC), OP.mult)
                    tt(vw(sxy[:], KC), vw(ub, KC), vw(axy[:], KC), OP.add)
                    act(vw(sq2[:], KC), vw(sxy[:], KC), AF.Square)
                    tt(nn[:, h0:h1], sq2[:, h0:h1], sq2[:, KC + h0:KC + h1],
                       OP.add)
                    ts(nnm[:, h0:h1], nn[:, h0:h1], 1.0, OP.max)
                    act(rho[:, h0:h1], nnm[:, h0:h1], AF.Abs_reciprocal_sqrt)
                    tt(vw(o_t, KC), vw(sxy[:], KC),
                       rho[:, h0:h1].rearrange(
                           "p (o b) -> p o b", o=1).broadcast_to([P, 2, w]),
                       OP.mult)
                    nc.sync.dma_start(
                        out=out_d[:].rearrange(
                            "p (i o b) -> p i o b", i=NT, o=2)[:, i, :, h0:h1],
                        in_=vw(o_t, KC))

            sts = {0: stage_dma(0)}
            nc.sync.dma_start(out=cst[:], in_=cst_in[:])
            stage_a(0, sts[0])
            for i in range(NT):
                if i + 1 < NT:
                    sts[i + 1] = stage_dma(i + 1)
                solve(i, sts[i])
                if i + 1 < NT:
                    stage_a(i + 1, sts[i + 1])
                solve2(i, sts.pop(i))
    nc.compile()
    return nc


def _get_nc():
    if "nc" not in _CACHE:
        _CACHE["nc"] = _build()
    return _CACHE["nc"]


def _run(u_nom: np.ndarray, obs: np.ndarray, trace: bool = False):
    from concourse.bass_utils import run_bass_kernel_spmd

    u_nom = np.asarray(u_nom, dtype=np.float32)
    obs = np.asarray(obs, dtype=np.float32)

    nc = _get_nc()
    in_maps = []
    for c in range(NCORES):
        s = slice(c * BC, (c + 1) * BC)
        uc = u_nom[s].reshape(P, NT, KC, 2).astype(bfloat16)
        oc = obs[s].reshape(P, NT, KC, 6)
        gx = (2.0 * oc[:, :, :, 2]).astype(bfloat16)
        gy = (2.0 * oc[:, :, :, 3]).astype(bfloat16)
        # clamp exact-zero Gx so S' = Gx^2+Gy^2 > 0 on device (no stt floor)
        gx = np.where(np.abs(gx.astype(np.float32)) < 1e-18,
                      np.float32(1e-18), gx.astype(np.float32)).astype(bfloat16)
        # blocks: [Gx | Gy | ux | uy | vx | vy], G = 2*p_rel
        pk = np.stack(
            [gx, gy,
             uc[:, :, :, 0], uc[:, :, :, 1],
             oc[:, :, :, 4].astype(bfloat16), oc[:, :, :, 5].astype(bfloat16)],
            axis=2).reshape(P, NPER * 6)
        cstv = np.tile(np.array([0.0, -1.0, 1.0, -0.5 * TOL, 1e-30,
                                 1e-38, 0.0, 0.0], dtype=np.float32), (P, 1))
        in_maps.append({"pk": np.ascontiguousarray(pk),
                        "cst": np.ascontiguousarray(cstv)})
    res = run_bass_kernel_spmd(nc, in_maps, core_ids=list(range(NCORES)),
                               trace=trace)
    out = np.empty((B, 2), dtype=np.float32)
    for c in range(NCORES):
        r = np.asarray(res.results[c]["out"]).view(bfloat16).astype(np.float32)
        r = r.reshape(P, NT, 2, KC)
        out[c * BC:(c + 1) * BC] = np.transpose(r, (0, 1, 3, 2)).reshape(BC, 2)
    return out, res


def kernel(u_nom: np.ndarray, obs: np.ndarray) -> np.ndarray:
    return _run(u_nom, obs)[0]


if __name__ == "__main__":
    rng = np.random.default_rng(0)
    u = rng.standard_normal((B, 2), dtype=np.float32)
    o = rng.standard_normal((B, 6), dtype=np.float32)
    r = kernel(u, o)
    print(r.shape, r.dtype, r[:4])
